# revision 19
# baseline (speedup 1.0000x reference)
"""Differential multi-head attention on 8 Trainium2 NeuronCores.

Sharding: core p owns head pair (p, p+8) for both batches (tensor parallel
over the 8 differential head pairs). lambda scalars are folded into the
output-projection weights on the host. Host sums the 8 partial outputs.

Layout per core (hd = 64, pair cols = 128, T = B*N = 4096 tokens):
  xT      [1024, 4096]   x transposed (features on partitions), fp16
  QT, KT  [128, 4096]    projected q/k transposed; rows 0:64 = head p,
                         rows 64:128 = head p+8
  V       [4096, 130]    token-partition layout, cols [h1(64) | 1 | h2(64) | 1]
  S.T     [k, q] chunks  via matmul(lhsT=KT slice, rhs=QT slice), K=64
  P.T     exp(S.T/8)     ACT, written as fp16
  OT_aug  [65, 512]      psum accum over 16 k-chunks: rows 0:64 = (P@V).T,
                         row 64 = softmax denominators
  out.T   [1024, 4096]   = Wcomb.T @ OcombT, partial (fp16); summed on host
"""
import numpy as np

import concourse.bacc as bacc
import concourse.bass as bass
import concourse.tile as tile
import concourse.mybir as mybir
from concourse.bass_utils import run_bass_kernel_spmd

F32 = mybir.dt.float32
F16 = mybir.dt.float16

EMBED = 1024
H2 = 8
HD = 64
B = 2
N = 2048
T = B * N  # 4096
NCORES = 8
LAMBDA_INIT = 0.8
SCALE = HD ** -0.5

TRACE = False
LAST_RESULT = [None]

_compiled = [None]


def ts(i, size):
    return slice(i * size, (i + 1) * size)


def _build():
    nc = bacc.Bacc("TRN2", target_bir_lowering=False, debug=False, num_devices=NCORES)

    xT_d = nc.dram_tensor("xT", [EMBED, T], F16, kind="ExternalInput").ap()
    wq_d = nc.dram_tensor("wq", [8, 128, 128], F16, kind="ExternalInput").ap()
    wk_d = nc.dram_tensor("wk", [8, 128, 128], F16, kind="ExternalInput").ap()
    wv_d = nc.dram_tensor("wv", [8, 128, 128], F16, kind="ExternalInput").ap()
    wc_d = nc.dram_tensor("wcomb", [128, 1024], F16, kind="ExternalInput").ap()
    bq_d = nc.dram_tensor("bq", [128, 1], F32, kind="ExternalInput").ap()
    bk_d = nc.dram_tensor("bk", [128, 1], F32, kind="ExternalInput").ap()
    bva_d = nc.dram_tensor("bvaug", [1, 130], F32, kind="ExternalInput").ap()
    outT_d = nc.dram_tensor("outT", [EMBED, T], F16, kind="ExternalOutput").ap()
    d_dram = nc.dram_tensor("d_scratch", [64, 512], F16).ap()
    rd_dram = nc.dram_tensor("rd_scratch", [64, 512], F16).ap()

    with tile.TileContext(nc) as tc:
        with (
            tc.tile_pool(name="consts", bufs=1) as consts,
            tc.tile_pool(name="xp", bufs=8) as xp,
            tc.tile_pool(name="qkv", bufs=1) as qkv,
            tc.tile_pool(name="ptp", bufs=2) as ptp,
            tc.tile_pool(name="stage", bufs=3) as stage,
            tc.tile_pool(name="bcp", bufs=2) as bcp,
            tc.tile_pool(name="outp", bufs=4) as outp,
            tc.tile_pool(name="ps_st", bufs=2, space="PSUM") as ps_st,
            tc.tile_pool(name="ps_ot", bufs=1, space="PSUM") as ps_ot,
            tc.tile_pool(name="ps_c", bufs=2, space="PSUM") as ps_c,
        ):
            # ---- load constants ----
            wq_t = consts.tile([128, 8, 128], F16, name="wq_t")
            wk_t = consts.tile([128, 8, 128], F16, name="wk_t")
            wv_t = consts.tile([128, 8, 128], F16, name="wv_t")
            wc_t = consts.tile([128, 1024], F16, name="wc_t")
            bq_t = consts.tile([128, 1], F32, name="bq_t")
            bk_t = consts.tile([128, 1], F32, name="bk_t")
            bva_t = consts.tile([128, 130], F32, name="bva_t")
            nc.sync.dma_start(out=wq_t, in_=wq_d.rearrange("c p m -> p c m"))
            nc.sync.dma_start(out=wk_t, in_=wk_d.rearrange("c p m -> p c m"))
            nc.sync.dma_start(out=wv_t, in_=wv_d.rearrange("c p m -> p c m"))
            nc.sync.dma_start(out=bq_t, in_=bq_d)
            nc.sync.dma_start(out=bk_t, in_=bk_d)
            nc.sync.dma_start(
                out=bva_t,
                in_=bass.AP(tensor=bva_d.tensor, offset=0,
                            ap=[[0, 128]] + list(bva_d.ap[-1:])),
            )

            qt_t = qkv.tile([128, T], F16, name="qt_t")
            kt_t = qkv.tile([128, T], F16, name="kt_t")
            v_t = qkv.tile([128, 32, 200], F16, name="v_t")
            ot_t = qkv.tile([128, B, N], F16, name="ot_t")
            oc_t = qkv.tile([128, B, N], F16, name="oc_t")

            xT_r = xT_d.rearrange("(c p) n -> p c n", p=128)

            xt_tiles = {}

            def xt_fetch(t):
                xt = xp.tile([128, 8, 512], F16, name="xt")
                nc.sync.dma_start(out=xt[:, 0:4, :], in_=xT_r[:, 0:4, ts(t, 512)])
                nc.scalar.dma_start(out=xt[:, 4:8, :], in_=xT_r[:, 4:8, ts(t, 512)])
                xt_tiles[t] = xt

            def proj_t(t):
                """Project token chunk t (512 tokens) -> QT/KT slices + V chunks."""
                xt = xt_tiles[t]
                for wt, dst, bias in ((wq_t, qt_t, bq_t), (wk_t, kt_t, bk_t)):
                    psq = ps_c.tile([128, 512], F32, name="ps_c")
                    for f in range(8):
                        nc.tensor.matmul(
                            psq, wt[:, f, :], xt[:, f, :],
                            start=(f == 0), stop=(f == 7),
                        )
                    nc.vector.tensor_scalar_add(dst[:, ts(t, 512)], psq, bias)
                for sub in range(4):
                    c = t * 4 + sub
                    psv = ps_c.tile([128, 512], F32, name="ps_c")
                    for f in range(8):
                        nc.tensor.matmul(
                            psv[:, 0:128], xt[:, f, ts(sub, 128)], wv_t[:, f, :],
                            start=(f == 0), stop=(f == 7),
                        )
                    nc.vector.tensor_add(v_t[:, c, 0:64], psv[:, 0:64], bva_t[:, 0:64])
                    nc.vector.tensor_add(v_t[:, c, 65:129], psv[:, 64:128], bva_t[:, 65:129])
                nc.vector.tensor_copy(
                    v_t[:, ts(t, 4), 64:65],
                    bva_t[:, None, 64:65].broadcast_to([128, 4, 1]),
                )
                nc.vector.tensor_copy(
                    v_t[:, ts(t, 4), 129:130],
                    bva_t[:, None, 129:130].broadcast_to([128, 4, 1]),
                )

            def drain_accums(b, qc, otps):
                """PSUM accumulators -> ot_t (SBUF) + denominator rows -> DRAM."""
                for h in (0, 1):
                    idx = b * 32 + qc * 2 + h
                    stg = stage.tile([65, 512], F16, name="stg")
                    nc.vector.tensor_copy(stg, otps[h][0:65, :])
                    nc.sync.dma_start(
                        out=ot_t[h * 64:(h + 1) * 64, b, ts(qc, 512)],
                        in_=stg[0:64, :],
                    )
                    nc.sync.dma_start(out=d_dram[idx:idx + 1, :], in_=stg[64:65, :])

            def norm1(b, qc):
                r0 = b * 32 + qc * 2
                d16 = bcp.tile([2, 512], F16, name="d16")
                d_b = bcp.tile([2, 512], F32, name="d_b")
                rd_b = bcp.tile([2, 512], F32, name="rd_b")
                rs_b = bcp.tile([2, 512], F32, name="rs_b")
                rd16 = bcp.tile([2, 512], F16, name="rd16")
                (nc.scalar if b == 1 and qc == 3 else nc.sync).dma_start(
                    out=d16, in_=d_dram[r0:r0 + 2, :])
                nc.vector.tensor_copy(d_b, d16)
                nc.vector.reciprocal_approx_accurate(rd_b, d_b, rs_b)
                nc.vector.tensor_copy(rd16, rd_b)
                (nc.scalar if b == 1 and qc == 3 else nc.sync).dma_start(
                    out=rd_dram[r0:r0 + 2, :], in_=rd16)

            def norm2(b, qc):
                r0 = b * 32 + qc * 2
                bc = bcp.tile([128, 512], F16, name="bc")
                for h in (0, 1):
                    nc.sync.dma_start(
                        out=bc[h * 64:(h + 1) * 64, :],
                        in_=bass.AP(tensor=rd_dram.tensor, offset=(r0 + h) * 512,
                                    ap=[[0, 64], [1, 512]]),
                    )
                nc.vector.tensor_mul(
                    oc_t[:, b, ts(qc, 512)], ot_t[:, b, ts(qc, 512)], bc
                )

            def outproj_m(b, qc, m):
                pso = ps_c.tile([128, 512], F32, name="ps_c")
                nc.tensor.matmul(
                    pso, wc_t[:, ts(m, 128)], oc_t[:, b, ts(qc, 512)],
                    start=True, stop=True,
                )
                so = outp.tile([128, 512], F16, name="so")
                nc.vector.tensor_copy(so, pso)
                nc.scalar.dma_start(
                    out=outT_d[ts(m, 128), b * N + qc * 512: b * N + (qc + 1) * 512],
                    in_=so,
                )

            nc.vector.memset(v_t[:, :, 130:200], 0.0)
            # prologue: prefetch all x chunks; project chunk 0; rest interleave
            for t in range(8):
                xt_fetch(t)
            proj_t(0)
            nc.sync.dma_start(out=wc_t, in_=wc_d)

            prev = None
            for b in range(2):
                for qc in range(4):
                    # filler work interleaved between attention groups:
                    filler = []
                    if b == 0 and qc == 0:
                        # remaining b=0 projections, gating kc availability:
                        # proj_t(kc//4 + 1) must precede kc group (kc//4+1)*4
                        pass
                    if prev is not None:
                        pb, pqc = prev
                        filler.append(lambda pb=pb, pqc=pqc: norm1(pb, pqc))
                        filler.append(lambda pb=pb, pqc=pqc: norm2(pb, pqc))
                        for m in range(8):
                            filler.append(
                                lambda pb=pb, pqc=pqc, m=m: outproj_m(pb, pqc, m))
                    if b == 0:
                        filler.append(lambda t=4 + qc: proj_t(t))

                    otps = [
                        ps_ot.tile([128, 512], F32, name=f"ps_ot{h}") for h in (0, 1)
                    ]
                    qoff = b * N + qc * 512
                    slots = [(kc, h) for kc in range(16) for h in (0, 1)]
                    for g0 in range(0, 32, 2):
                        grp = slots[g0:g0 + 2]
                        st = ps_st.tile([128, 1024], F32, name="ps_st")
                        pt = ptp.tile([128, 1024], F16, name="pt")
                        for i, (kc, h) in enumerate(grp):
                            lo = h * 64
                            koff = b * N + kc * 128
                            nc.tensor.matmul(
                                st[:, ts(i, 512)],
                                kt_t[lo:lo + 64, koff:koff + 128],
                                qt_t[lo:lo + 64, qoff:qoff + 512],
                                start=True, stop=True,
                            )
                        w = len(grp) * 512
                        nc.scalar.activation(
                            pt[:, 0:w], st[:, 0:w],
                            mybir.ActivationFunctionType.Exp, scale=SCALE,
                        )
                        for i, (kc, h) in enumerate(grp):
                            nc.tensor.matmul(
                                otps[h],
                                v_t[:, b * 16 + kc, h * 65:h * 65 + 128],
                                pt[:, ts(i, 512)],
                                start=(kc == 0), stop=(kc == 15),
                            )
                        if b == 0 and qc == 0 and g0 in (6, 14, 22):
                            # feed the next projection chunk just in time for
                            # the k-chunks that need it (kc group 4/8/12)
                            proj_t(g0 // 8 + 1)
                        last = (b == 1 and qc == 3)
                        if filler and g0 >= 2 and (not last or g0 <= 8):
                            filler.pop(0)()
                    drain_accums(b, qc, otps)
                    leftover = filler
                    prev = (b, qc)

            # epilogue: last chunk's chain interleaved with deferred work
            norm1(1, 3)
            for f in leftover[:6]:
                f()
            norm2(1, 3)
            for f in leftover[6:]:
                f()
            for m in range(8):
                outproj_m(1, 3, m)

    nc.compile()
    return nc


def kernel(x, Wq, bq, Wk, bk, Wv, bv, Wp, bp,
           lambda_q1, lambda_k1, lambda_q2, lambda_k2):
    x = np.asarray(x, dtype=np.float32)
    Wq, Wk, Wv, Wp = [np.asarray(w, dtype=np.float32) for w in (Wq, Wk, Wv, Wp)]
    bq, bk, bv, bp = [np.asarray(v, dtype=np.float32) for v in (bq, bk, bv, bp)]

    l1 = np.exp(np.minimum(
        (np.asarray(lambda_q1, np.float32) * np.asarray(lambda_k1, np.float32))
        .sum((-1, -2)), 5.0))
    l2 = np.exp(np.minimum(
        (np.asarray(lambda_q2, np.float32) * np.asarray(lambda_k2, np.float32))
        .sum((-1, -2)), 5.0))
    lv = np.float32((l1 - l2 + np.float32(LAMBDA_INIT)).mean())

    xT = np.ascontiguousarray(x.reshape(T, EMBED).T.astype(np.float16))

    if _compiled[0] is None:
        _compiled[0] = _build()
    nc = _compiled[0]

    in_maps = []
    for p in range(NCORES):
        r1 = slice(p * HD, (p + 1) * HD)          # head p rows/cols
        r2 = slice((8 + p) * HD, (9 + p) * HD)    # head p+8 rows/cols
        wq_p = np.concatenate([Wq[r1], Wq[r2]], 0).T      # [1024, 128]
        wk_p = np.concatenate([Wk[r1], Wk[r2]], 0).T
        wv_p = np.concatenate([Wv[r1], Wv[r2]], 0).T
        wpt1 = Wp[:, r1].T                                 # [64, 1024]
        wpt2 = Wp[:, r2].T
        wcomb = np.concatenate([wpt1, wpt2 - lv * wpt1], 0)  # [128, 1024]
        bva = np.concatenate(
            [bv[r1], [1.0], bv[r2], [1.0]]).astype(np.float32)[None, :]
        in_maps.append({
            "xT": xT,
            "wq": np.ascontiguousarray(wq_p.reshape(8, 128, 128).astype(np.float16)),
            "wk": np.ascontiguousarray(wk_p.reshape(8, 128, 128).astype(np.float16)),
            "wv": np.ascontiguousarray(wv_p.reshape(8, 128, 128).astype(np.float16)),
            "wcomb": np.ascontiguousarray(wcomb.astype(np.float16)),
            "bq": np.concatenate([bq[r1], bq[r2]])[:, None].copy(),
            "bk": np.concatenate([bk[r1], bk[r2]])[:, None].copy(),
            "bvaug": np.ascontiguousarray(bva),
        })

    res = run_bass_kernel_spmd(
        nc, in_maps, core_ids=list(range(NCORES)), trace=TRACE,
    )
    LAST_RESULT[0] = res

    outT = res.results[0]["outT"].astype(np.float64)
    for c in range(1, NCORES):
        outT += res.results[c]["outT"]
    out = outT.T.reshape(B, N, EMBED).astype(np.float32) + bp[None, None, :]
    return out


# revision 20
# speedup vs baseline: 1.0452x; 1.0452x over previous
"""Differential multi-head attention on 8 Trainium2 NeuronCores.

Sharding: core p owns head pair (p, p+8) for both batches (tensor parallel
over the 8 differential head pairs). lambda scalars are folded into the
output-projection weights on the host. Host sums the 8 partial outputs.

Layout per core (hd = 64, pair cols = 128, T = B*N = 4096 tokens):
  xT      [1024, 4096]   x transposed (features on partitions), fp16
  QT, KT  [128, 4096]    projected q/k transposed; rows 0:64 = head p,
                         rows 64:128 = head p+8
  V       [4096, 130]    token-partition layout, cols [h1(64) | 1 | h2(64) | 1]
  S.T     [k, q] chunks  via matmul(lhsT=KT slice, rhs=QT slice), K=64
  P.T     exp(S.T/8)     ACT, written as fp16
  OT_aug  [65, 512]      psum accum over 16 k-chunks: rows 0:64 = (P@V).T,
                         row 64 = softmax denominators
  out.T   [1024, 4096]   = Wcomb.T @ OcombT, partial (fp16); summed on host
"""
import numpy as np

import concourse.bacc as bacc
import concourse.bass as bass
import concourse.tile as tile
import concourse.mybir as mybir
from concourse.bass_utils import run_bass_kernel_spmd

F32 = mybir.dt.float32
F16 = mybir.dt.float16

EMBED = 1024
H2 = 8
HD = 64
B = 2
N = 2048
T = B * N  # 4096
NCORES = 8
LAMBDA_INIT = 0.8
SCALE = HD ** -0.5

TRACE = False
LAST_RESULT = [None]

_compiled = [None]


def ts(i, size):
    return slice(i * size, (i + 1) * size)


def _build():
    nc = bacc.Bacc("TRN2", target_bir_lowering=False, debug=False, num_devices=NCORES)

    xT_d = nc.dram_tensor("xT", [EMBED, T], F16, kind="ExternalInput").ap()
    wq_d = nc.dram_tensor("wq", [8, 128, 128], F16, kind="ExternalInput").ap()
    wk_d = nc.dram_tensor("wk", [8, 128, 128], F16, kind="ExternalInput").ap()
    wv_d = nc.dram_tensor("wv", [8, 128, 128], F16, kind="ExternalInput").ap()
    wc_d = nc.dram_tensor("wcomb", [128, 1024], F16, kind="ExternalInput").ap()
    bq_d = nc.dram_tensor("bq", [128, 1], F32, kind="ExternalInput").ap()
    bk_d = nc.dram_tensor("bk", [128, 1], F32, kind="ExternalInput").ap()
    bva_d = nc.dram_tensor("bvaug", [1, 130], F32, kind="ExternalInput").ap()
    outT_d = nc.dram_tensor("outT", [EMBED, T], F16, kind="ExternalOutput").ap()
    d_dram = nc.dram_tensor("d_scratch", [64, 512], F16).ap()
    rd_dram = nc.dram_tensor("rd_scratch", [64, 512], F16).ap()

    with tile.TileContext(nc) as tc:
        with (
            tc.tile_pool(name="consts", bufs=1) as consts,
            tc.tile_pool(name="xp", bufs=8) as xp,
            tc.tile_pool(name="qkv", bufs=1) as qkv,
            tc.tile_pool(name="ptp", bufs=2) as ptp,
            tc.tile_pool(name="stage", bufs=3) as stage,
            tc.tile_pool(name="bcp", bufs=2) as bcp,
            tc.tile_pool(name="outp", bufs=4) as outp,
            tc.tile_pool(name="ps_st", bufs=2, space="PSUM") as ps_st,
            tc.tile_pool(name="ps_ot", bufs=1, space="PSUM") as ps_ot,
            tc.tile_pool(name="ps_c", bufs=2, space="PSUM") as ps_c,
        ):
            # ---- load constants ----
            wq_t = consts.tile([128, 8, 128], F16, name="wq_t")
            wk_t = consts.tile([128, 8, 128], F16, name="wk_t")
            wv_t = consts.tile([128, 8, 128], F16, name="wv_t")
            wc_t = consts.tile([128, 1024], F16, name="wc_t")
            bq_t = consts.tile([128, 1], F32, name="bq_t")
            bk_t = consts.tile([128, 1], F32, name="bk_t")
            bva_t = consts.tile([128, 130], F32, name="bva_t")
            nc.sync.dma_start(out=wq_t, in_=wq_d.rearrange("c p m -> p c m"))
            nc.sync.dma_start(out=wk_t, in_=wk_d.rearrange("c p m -> p c m"))
            nc.sync.dma_start(out=wv_t, in_=wv_d.rearrange("c p m -> p c m"))
            nc.sync.dma_start(out=bq_t, in_=bq_d)
            nc.sync.dma_start(out=bk_t, in_=bk_d)
            nc.sync.dma_start(
                out=bva_t,
                in_=bass.AP(tensor=bva_d.tensor, offset=0,
                            ap=[[0, 128]] + list(bva_d.ap[-1:])),
            )

            qt_t = qkv.tile([128, T], F16, name="qt_t")
            kt_t = qkv.tile([128, T], F16, name="kt_t")
            v_t = qkv.tile([128, 32, 200], F16, name="v_t")
            ot_t = qkv.tile([128, B, N], F16, name="ot_t")
            oc_t = qkv.tile([128, B, N], F16, name="oc_t")

            xT_r = xT_d.rearrange("(c p) n -> p c n", p=128)

            xt_tiles = {}

            def xt_fetch(t):
                xt = xp.tile([128, 8, 512], F16, name="xt")
                nc.sync.dma_start(out=xt, in_=xT_r[:, :, ts(t, 512)])
                xt_tiles[t] = xt

            def proj_t(t):
                """Project token chunk t (512 tokens) -> QT/KT slices + V chunks."""
                xt = xt_tiles[t]
                for wt, dst, bias in ((wq_t, qt_t, bq_t), (wk_t, kt_t, bk_t)):
                    psq = ps_c.tile([128, 512], F32, name="ps_c")
                    for f in range(8):
                        nc.tensor.matmul(
                            psq, wt[:, f, :], xt[:, f, :],
                            start=(f == 0), stop=(f == 7),
                        )
                    nc.vector.tensor_scalar_add(dst[:, ts(t, 512)], psq, bias)
                for sub in range(4):
                    c = t * 4 + sub
                    psv = ps_c.tile([128, 512], F32, name="ps_c")
                    for f in range(8):
                        nc.tensor.matmul(
                            psv[:, 0:128], xt[:, f, ts(sub, 128)], wv_t[:, f, :],
                            start=(f == 0), stop=(f == 7),
                        )
                    nc.vector.tensor_add(v_t[:, c, 0:64], psv[:, 0:64], bva_t[:, 0:64])
                    nc.vector.tensor_add(v_t[:, c, 65:129], psv[:, 64:128], bva_t[:, 65:129])
                nc.vector.tensor_copy(
                    v_t[:, ts(t, 4), 64:65],
                    bva_t[:, None, 64:65].broadcast_to([128, 4, 1]),
                )
                nc.vector.tensor_copy(
                    v_t[:, ts(t, 4), 129:130],
                    bva_t[:, None, 129:130].broadcast_to([128, 4, 1]),
                )

            def drain_accums(b, qc, otps):
                """PSUM accumulators -> ot_t (SBUF) + denominator rows -> DRAM."""
                for h in (0, 1):
                    idx = b * 32 + qc * 2 + h
                    stg = stage.tile([65, 512], F16, name="stg")
                    nc.vector.tensor_copy(stg, otps[h][0:65, :])
                    nc.sync.dma_start(
                        out=ot_t[h * 64:(h + 1) * 64, b, ts(qc, 512)],
                        in_=stg[0:64, :],
                    )
                    nc.sync.dma_start(out=d_dram[idx:idx + 1, :], in_=stg[64:65, :])

            def norm1(b, qc):
                r0 = b * 32 + qc * 2
                d16 = bcp.tile([2, 512], F16, name="d16")
                d_b = bcp.tile([2, 512], F32, name="d_b")
                rd_b = bcp.tile([2, 512], F32, name="rd_b")
                rs_b = bcp.tile([2, 512], F32, name="rs_b")
                rd16 = bcp.tile([2, 512], F16, name="rd16")
                nc.sync.dma_start(out=d16, in_=d_dram[r0:r0 + 2, :])
                nc.vector.tensor_copy(d_b, d16)
                nc.vector.reciprocal_approx_accurate(rd_b, d_b, rs_b)
                nc.vector.tensor_copy(rd16, rd_b)
                nc.sync.dma_start(out=rd_dram[r0:r0 + 2, :], in_=rd16)

            def norm2(b, qc):
                r0 = b * 32 + qc * 2
                bc = bcp.tile([128, 512], F16, name="bc")
                for h in (0, 1):
                    nc.sync.dma_start(
                        out=bc[h * 64:(h + 1) * 64, :],
                        in_=bass.AP(tensor=rd_dram.tensor, offset=(r0 + h) * 512,
                                    ap=[[0, 64], [1, 512]]),
                    )
                nc.vector.tensor_mul(
                    oc_t[:, b, ts(qc, 512)], ot_t[:, b, ts(qc, 512)], bc
                )

            def outproj_m(b, qc, m):
                pso = ps_c.tile([128, 512], F32, name="ps_c")
                nc.tensor.matmul(
                    pso, wc_t[:, ts(m, 128)], oc_t[:, b, ts(qc, 512)],
                    start=True, stop=True,
                )
                so = outp.tile([128, 512], F16, name="so")
                nc.vector.tensor_copy(so, pso)
                nc.scalar.dma_start(
                    out=outT_d[ts(m, 128), b * N + qc * 512: b * N + (qc + 1) * 512],
                    in_=so,
                )

            nc.vector.memset(v_t[:, :, 130:200], 0.0)
            # prologue: prefetch all x chunks; project chunk 0; rest interleave
            for t in range(8):
                xt_fetch(t)
            proj_t(0)
            nc.sync.dma_start(out=wc_t, in_=wc_d)

            prev = None
            for b in range(2):
                for qc in range(4):
                    # filler work interleaved between attention groups:
                    filler = []
                    if b == 0 and qc == 0:
                        # remaining b=0 projections, gating kc availability:
                        # proj_t(kc//4 + 1) must precede kc group (kc//4+1)*4
                        pass
                    if prev is not None:
                        pb, pqc = prev
                        filler.append(lambda pb=pb, pqc=pqc: norm1(pb, pqc))
                        filler.append(lambda pb=pb, pqc=pqc: norm2(pb, pqc))
                        for m in range(8):
                            filler.append(
                                lambda pb=pb, pqc=pqc, m=m: outproj_m(pb, pqc, m))
                    if b == 0:
                        filler.append(lambda t=4 + qc: proj_t(t))

                    otps = [
                        ps_ot.tile([128, 512], F32, name=f"ps_ot{h}") for h in (0, 1)
                    ]
                    qoff = b * N + qc * 512
                    slots = [(kc, h) for kc in range(16) for h in (0, 1)]
                    pending = None  # PV work delayed one group (PE FIFO overlap)
                    for g0 in range(0, 34, 2):
                        if g0 < 32:
                            grp = slots[g0:g0 + 2]
                            st = ps_st.tile([128, 1024], F32, name="ps_st")
                            pt = ptp.tile([128, 1024], F16, name="pt")
                            for i, (kc, h) in enumerate(grp):
                                lo = h * 64
                                koff = b * N + kc * 128
                                nc.tensor.matmul(
                                    st[:, ts(i, 512)],
                                    kt_t[lo:lo + 64, koff:koff + 128],
                                    qt_t[lo:lo + 64, qoff:qoff + 512],
                                    start=True, stop=True,
                                )
                            nc.scalar.activation(
                                pt[:, 0:len(grp) * 512], st[:, 0:len(grp) * 512],
                                mybir.ActivationFunctionType.Exp, scale=SCALE,
                            )
                        if pending is not None:
                            pgrp, ppt = pending
                            for i, (kc, h) in enumerate(pgrp):
                                nc.tensor.matmul(
                                    otps[h],
                                    v_t[:, b * 16 + kc, h * 65:h * 65 + 128],
                                    ppt[:, ts(i, 512)],
                                    start=(kc == 0), stop=(kc == 15),
                                )
                        pending = (grp, pt) if g0 < 32 else None
                        if b == 0 and qc == 0 and g0 in (6, 14, 22):
                            # feed the next projection chunk just in time for
                            # the k-chunks that need it (kc group 4/8/12)
                            proj_t(g0 // 8 + 1)
                        last = (b == 1 and qc == 3)
                        if filler and g0 >= 2 and (not last or g0 <= 8):
                            filler.pop(0)()
                    drain_accums(b, qc, otps)
                    leftover = filler
                    prev = (b, qc)

            # epilogue: last chunk's chain interleaved with deferred work
            norm1(1, 3)
            for f in leftover[:6]:
                f()
            norm2(1, 3)
            for f in leftover[6:]:
                f()
            for m in range(8):
                outproj_m(1, 3, m)

    nc.compile()
    return nc


def kernel(x, Wq, bq, Wk, bk, Wv, bv, Wp, bp,
           lambda_q1, lambda_k1, lambda_q2, lambda_k2):
    x = np.asarray(x, dtype=np.float32)
    Wq, Wk, Wv, Wp = [np.asarray(w, dtype=np.float32) for w in (Wq, Wk, Wv, Wp)]
    bq, bk, bv, bp = [np.asarray(v, dtype=np.float32) for v in (bq, bk, bv, bp)]

    l1 = np.exp(np.minimum(
        (np.asarray(lambda_q1, np.float32) * np.asarray(lambda_k1, np.float32))
        .sum((-1, -2)), 5.0))
    l2 = np.exp(np.minimum(
        (np.asarray(lambda_q2, np.float32) * np.asarray(lambda_k2, np.float32))
        .sum((-1, -2)), 5.0))
    lv = np.float32((l1 - l2 + np.float32(LAMBDA_INIT)).mean())

    xT = np.ascontiguousarray(x.reshape(T, EMBED).T.astype(np.float16))

    if _compiled[0] is None:
        _compiled[0] = _build()
    nc = _compiled[0]

    in_maps = []
    for p in range(NCORES):
        r1 = slice(p * HD, (p + 1) * HD)          # head p rows/cols
        r2 = slice((8 + p) * HD, (9 + p) * HD)    # head p+8 rows/cols
        wq_p = np.concatenate([Wq[r1], Wq[r2]], 0).T      # [1024, 128]
        wk_p = np.concatenate([Wk[r1], Wk[r2]], 0).T
        wv_p = np.concatenate([Wv[r1], Wv[r2]], 0).T
        wpt1 = Wp[:, r1].T                                 # [64, 1024]
        wpt2 = Wp[:, r2].T
        wcomb = np.concatenate([wpt1, wpt2 - lv * wpt1], 0)  # [128, 1024]
        bva = np.concatenate(
            [bv[r1], [1.0], bv[r2], [1.0]]).astype(np.float32)[None, :]
        in_maps.append({
            "xT": xT,
            "wq": np.ascontiguousarray(wq_p.reshape(8, 128, 128).astype(np.float16)),
            "wk": np.ascontiguousarray(wk_p.reshape(8, 128, 128).astype(np.float16)),
            "wv": np.ascontiguousarray(wv_p.reshape(8, 128, 128).astype(np.float16)),
            "wcomb": np.ascontiguousarray(wcomb.astype(np.float16)),
            "bq": np.concatenate([bq[r1], bq[r2]])[:, None].copy(),
            "bk": np.concatenate([bk[r1], bk[r2]])[:, None].copy(),
            "bvaug": np.ascontiguousarray(bva),
        })

    res = run_bass_kernel_spmd(
        nc, in_maps, core_ids=list(range(NCORES)), trace=TRACE,
    )
    LAST_RESULT[0] = res

    outT = res.results[0]["outT"].astype(np.float64)
    for c in range(1, NCORES):
        outT += res.results[c]["outT"]
    out = outT.T.reshape(B, N, EMBED).astype(np.float32) + bp[None, None, :]
    return out
